# revision 14
# baseline (speedup 1.0000x reference)
"""Causal single-head attention (B=4, S=2048, D=1024) on 8 TRN2 NeuronCores.

Sharding: 2 cores per batch; each core owns 8 q-blocks of 128 rows chosen so
both cores of a batch see the same multiset of causal kv-span lengths
(padded to 512-chunks): core h=0 -> q-blocks [0,3,4,7,8,11,12,15],
core h=1 -> [1,2,5,6,9,10,13,14]; both give span chunks [1,1,2,2,3,3,4,4].
This makes one SPMD program valid for all 8 cores; per-core differences
(which q rows, causal mask offsets) ride in the input data.

Math per core (all matmuls in float32r, fp32 accumulation):
  Q^T = (Wq/sqrt(D))^T @ qT,  K^T = Wk^T @ kT        (projections)
  S_i = Q_i^T.T @ K^T (+ additive causal mask)        (scores per q-block)
  P = exp(S), denom = rowsum(P)                       (no max-sub: |S| < ~10)
  T_i = (P @ v) / denom                               (reassociated: raw v!)
  out_i = T_i @ Wv                                    (deferred out-proj)
Reassociation (P@v)@Wv replaces attn@(v@Wv) - saves the V projection.
"""

import os

import numpy as np

import concourse.bass as bass
import concourse.mybir as mybir
import concourse.tile as tile
from concourse import bacc
from concourse.bass_utils import run_bass_kernel_spmd

B, S, D = 4, 2048, 1024
P = 128                      # partitions / q-block rows
NBLK = 8                     # q-blocks per core
CH = 512                     # kv chunk (matmul moving free dim)
NCH = [1, 1, 2, 2, 3, 3, 4, 4]   # kv 512-chunks per q-block position
ORDER_A = [0, 2, 4, 5, 6, 7]    # first segment: needs all of v
ORDER_B = [3, 1]                # tail segment: only v chunks 0-7; frees SBUF
                                # so phase-4 inputs (wv, tt) prefetch under it
BLOCKS = [[0, 3, 4, 7, 8, 11, 12, 15], [1, 2, 5, 6, 9, 10, 13, 14]]
MASK_BASE = [[0, 384], [128, 256]]   # base[h][pos % 2]: col c allowed iff c <= base + r
DT = mybir.dt.float32r
F32 = mybir.dt.float32
NEG = -1e30

_cached = {}


def _build():
    if "nc" in _cached:
        return _cached["nc"]
    nc = bacc.Bacc("TRN2", target_bir_lowering=False, debug=False, num_devices=8)
    qT = nc.dram_tensor("qT", [D, P * NBLK], DT, kind="ExternalInput").ap()
    kTh = nc.dram_tensor("kTh", [D, S // 2], DT, kind="ExternalInput").ap()
    v = nc.dram_tensor("v", [S, D], DT, kind="ExternalInput").ap()
    wq = nc.dram_tensor("wq", [D, D], DT, kind="ExternalInput").ap()
    wk = nc.dram_tensor("wk", [D, D], DT, kind="ExternalInput").ap()
    wv = nc.dram_tensor("wv", [D, D], DT, kind="ExternalInput").ap()
    mask = nc.dram_tensor("mask", [P, 2, CH], F32, kind="ExternalInput").ap()
    ident = nc.dram_tensor("ident", [P, P], DT, kind="ExternalInput").ap()
    out = nc.dram_tensor("out", [P * NBLK, D], F32, kind="ExternalOutput").ap()

    KO = D // P      # 8 contraction chunks
    NV = S // P      # 16 v row-chunks
    QK = S // 4      # 512-col staging quarters of the kv-half input

    kTh_r = kTh.rearrange("(ko p) s -> p ko s", p=P)

    with tile.TileContext(nc) as tc:
        with tc.tile_pool(name="pers", bufs=1) as pers, \
             tc.tile_pool(name="dram", bufs=1, space="DRAM") as dpool:
            ident_sb = pers.tile([P, P], DT)
            nc.sync.dma_start(ident_sb[:], ident)
            mask_sb = pers.tile([P, 2, CH], F32)
            nc.sync.dma_start(mask_sb[:], mask)
            QT_sb = pers.tile([P, KO, P * NBLK], DT)
            KT_sb = pers.tile([P, KO, S], DT)
            tt_dram = [dpool.tile([P, D], DT, name=f"ttd_{i}") for i in range(NBLK)]
            cc_in = dpool.tile([P, KO, S // 2], DT)
            cc_out = dpool.tile([2, P, KO, S // 2], DT)

            # ---- Phase 1: K projection of OWN kv-half (other half comes via
            #      pairwise AllGather, hidden under Q-projection) ----
            wk_r = wk.rearrange("(ko p) m -> p ko m", p=P)
            wk_h = [None] * 2

            def load_wk_half(pool, hh):
                t = pool.tile([P, KO, D // 2], DT, name=f"wk_{hh}")
                for ko in range(KO):
                    nc.sync.dma_start(
                        t[:, ko], wk_r[:, ko, hh * (D // 2):(hh + 1) * (D // 2)])
                wk_h[hh] = t

            with tc.tile_pool(name="qtpre", bufs=1) as qtpre, \
                 tc.tile_pool(name="ps_proj", bufs=4, space="PSUM") as psp:
                qT_sb = qtpre.tile([P, KO, P * NBLK], DT)
                with tc.tile_pool(name="wkpool0", bufs=1) as wkpool0, \
                     tc.tile_pool(name="wkpool1", bufs=1) as wkpool1, \
                     tc.tile_pool(name="kstageA", bufs=1) as kstageA, \
                     tc.tile_pool(name="kstageB", bufs=1) as kstageB, \
                     tc.tile_pool(name="kout", bufs=3) as kout:
                    load_wk_half(wkpool0, 0)
                    kts = []
                    for sq, kpool in ((0, kstageA), (1, kstageB)):
                        t = kpool.tile([P, KO, QK], DT, name=f"kts_{sq}")
                        for ko in range(KO):
                            nc.sync.dma_start(
                                t[:, ko], kTh_r[:, ko, sq * QK:(sq + 1) * QK])
                        kts.append(t)
                    load_wk_half(wkpool1, 1)
                    for ko in range(KO):
                        nc.sync.dma_start(
                            qT_sb[:, ko],
                            qT.rearrange("(ko p) s -> p ko s", p=P)[:, ko])
                    for sq in range(2):
                        for m in range(KO):
                            ps = psp.tile([P, CH], F32, tag="kp", name=f"kp_{sq}_{m}")
                            for k in range(KO):
                                nc.tensor.matmul(
                                    ps[:], wk_h[m // 4][:, k, bass.ts(m % 4, P)],
                                    kts[sq][:, k, 0:CH],
                                    start=(k == 0), stop=(k == KO - 1))
                            ko_sb = kout.tile([P, CH], DT, tag="ko")
                            nc.vector.tensor_copy(ko_sb[:], ps[:])
                            nc.sync.dma_start(
                                cc_in[:, m, bass.ts(sq, CH)], ko_sb[:])

                nc.gpsimd.collective_compute(
                    "AllGather", mybir.AluOpType.bypass,
                    replica_groups=[[0, 1], [2, 3], [4, 5], [6, 7]],
                    ins=[cc_in.opt()], outs=[cc_out.opt()])

                # ---- Phase 2: Q projection; gathered K^T streams into SBUF
                #      underneath it (kv chunks 0,1 first) ----
                with tc.tile_pool(name="qproj", bufs=1) as qpool:
                    wq_sb = qpool.tile([P, KO, D], DT)
                    for ko in range(KO):
                        nc.sync.dma_start(
                            wq_sb[:, ko],
                            wq.rearrange("(ko p) m -> p ko m", p=P)[:, ko])
                    for c in range(2):
                        for m in range(KO):
                            nc.sync.dma_start(
                                KT_sb[:, m, bass.ts(c, CH)],
                                cc_out[c // 2, :, m, bass.ts(c % 2, CH)])
                    for m in range(KO):
                        for n in range(2):
                            ps = psp.tile([P, CH], F32, tag="pp")
                            for k in range(KO):
                                nc.tensor.matmul(
                                    ps[:], wq_sb[:, k, bass.ts(m, P)],
                                    qT_sb[:, k, bass.ts(n, CH)],
                                    start=(k == 0), stop=(k == KO - 1))
                            nc.vector.tensor_copy(QT_sb[:, m, bass.ts(n, CH)], ps[:])

            # ---- Phase 3: attention per q-block; T spilled to DRAM.
            #      Pipelined: scores run one chunk ahead of transpose+AV. ----
            wv_r = wv.rearrange("(ko p) m -> p ko m", p=P)
            v_r = v.rearrange("(so p) d -> p so d", p=P)
            if True:
                with tc.tile_pool(name="vlo", bufs=1) as vlo_pool, \
                     tc.tile_pool(name="cwork", bufs=2) as cwork, \
                     tc.tile_pool(name="ppool", bufs=3) as ppool, \
                     tc.tile_pool(name="ptpool", bufs=4) as ptpool, \
                     tc.tile_pool(name="ps_s", bufs=3, space="PSUM") as ps_s, \
                     tc.tile_pool(name="ps_tr", bufs=2, space="PSUM") as ps_tr, \
                     tc.tile_pool(name="ps_t", bufs=1, space="PSUM") as ps_t:
                    v_lo = vlo_pool.tile([P, NV // 2, D], DT)
                    for so in range(NV // 2):
                        nc.sync.dma_start(v_lo[:, so], v_r[:, so])

                    def v_chunk(kvi):
                        if kvi < NV // 2:
                            return v_lo[:, kvi]
                        return v_hi[:, kvi - NV // 2]

                    def attention_block(i):
                        nch = NCH[i]
                        nkv = nch * (CH // P)
                        ps_T0 = ps_t.tile([P, CH], F32, tag="T0",
                                          name=f"T0_{i}")
                        ps_T1 = ps_t.tile([P, CH], F32, tag="T1",
                                          name=f"T1_{i}")
                        dsums = []
                        p_tiles = []

                        def emit_scores(c, i=i, nch=nch):
                            ps_c = ps_s.tile([P, CH], F32, tag="s",
                                             name=f"s_{i}_{c}")
                            for k in range(KO):
                                nc.tensor.matmul(
                                    ps_c[:], QT_sb[:, k, bass.ts(i, P)],
                                    KT_sb[:, k, bass.ts(c, CH)],
                                    start=(k == 0), stop=(k == KO - 1))
                            if c == nch - 1:
                                nc.vector.tensor_tensor(
                                    ps_c[:], ps_c[:], mask_sb[:, i % 2],
                                    mybir.AluOpType.add)
                            p_sb = ppool.tile([P, CH], DT, tag="p",
                                              name=f"p_{i}_{c}")
                            ds = cwork.tile([P, 1], F32, tag="ds",
                                            name=f"ds_{i}_{c}")
                            nc.scalar.activation(
                                p_sb[:], ps_c[:],
                                mybir.ActivationFunctionType.Exp, accum_out=ds[:])
                            dsums.append(ds)
                            p_tiles.append(p_sb)

                        def emit_trav(c, i=i, nkv=nkv):
                            # transposes run 2 ahead of the AV matmuls
                            pts = []
                            for t in range(CH // P):
                                ptr = ps_tr.tile([P, P], DT, tag="tr")
                                nc.tensor.transpose(
                                    ptr[:], p_tiles[c][:, bass.ts(t, P)],
                                    ident_sb[:])
                                pt_sb = ptpool.tile([P, P], DT, tag="pt")
                                nc.vector.tensor_copy(pt_sb[:], ptr[:])
                                pts.append(pt_sb)
                                if t >= 2:
                                    _emit_av(c, t - 2, pts[t - 2], i, nkv)
                            _emit_av(c, 2, pts[2], i, nkv)
                            _emit_av(c, 3, pts[3], i, nkv)

                        def _emit_av(c, t, pt_sb, i, nkv):
                            kvi = c * (CH // P) + t
                            vc = v_chunk(kvi)
                            nc.tensor.matmul(
                                ps_T0[:], pt_sb[:], vc[:, 0:CH],
                                start=(kvi == 0), stop=(kvi == nkv - 1))
                            nc.tensor.matmul(
                                ps_T1[:], pt_sb[:], vc[:, CH:D],
                                start=(kvi == 0), stop=(kvi == nkv - 1))

                        for c in range(nch):
                            emit_scores(c)
                            if c >= 1:
                                emit_trav(c - 1)
                        emit_trav(nch - 1)

                        denom = cwork.tile([P, 1], F32, tag="den")
                        if nch == 1:
                            nc.vector.tensor_copy(denom[:], dsums[0][:])
                        else:
                            nc.vector.tensor_tensor(
                                denom[:], dsums[0][:], dsums[1][:],
                                mybir.AluOpType.add)
                            for c in range(2, nch):
                                nc.vector.tensor_tensor(
                                    denom[:], denom[:], dsums[c][:],
                                    mybir.AluOpType.add)
                        rden = cwork.tile([P, 1], F32, tag="rden")
                        nc.vector.reciprocal(rden[:], denom[:])
                        t_st = cwork.tile([P, D], DT, tag="tst", bufs=1)
                        nc.vector.tensor_scalar_mul(t_st[:, 0:CH], ps_T0[:], rden[:])
                        nc.vector.tensor_scalar_mul(t_st[:, CH:D], ps_T1[:], rden[:])
                        tt_st = cwork.tile([P, KO, P], DT, tag="ttst")
                        for d in range(KO):
                            ptr = ps_tr.tile([P, P], DT, tag="tr")
                            nc.tensor.transpose(
                                ptr[:], t_st[:, bass.ts(d, P)], ident_sb[:])
                            nc.vector.tensor_copy(tt_st[:, d], ptr[:])
                        nc.sync.dma_start(tt_dram[i][:], tt_st[:])

                    for m in range(KO):
                        nc.sync.dma_start(
                            KT_sb[:, m, bass.ts(2, CH)],
                            cc_out[1, :, m, bass.ts(0, CH)])
                    with tc.tile_pool(name="vhi", bufs=1) as vhi_pool:
                        v_hi = vhi_pool.tile([P, NV // 2, D], DT)
                        for so in range(NV // 2):
                            nc.sync.dma_start(v_hi[:, so], v_r[:, NV // 2 + so])
                        for m in range(KO):
                            nc.sync.dma_start(
                                KT_sb[:, m, bass.ts(3, CH)],
                                cc_out[1, :, m, bass.ts(1, CH)])
                        for i in ORDER_A:
                            attention_block(i)
                    # v_hi freed: prefetch phase-4 inputs under the tail blocks
                    with tc.tile_pool(name="wvpool", bufs=1) as wvpool:
                        wv_sb = wvpool.tile([P, KO, D], DT)
                        for ko in range(KO):
                            nc.sync.dma_start(wv_sb[:, ko], wv_r[:, ko])
                        for i in ORDER_B:
                            attention_block(i)

                        # ---- Phase 4: out = TT.T @ Wv (pure matmuls) ----
                        with tc.tile_pool(name="dwork", bufs=2) as dwork:
                            for i in range(NBLK):
                                tt_rd = dwork.tile([P, KO, P], DT, tag="ttrd")
                                nc.sync.dma_start(tt_rd[:], tt_dram[i][:])
                                ps_o0 = ps_t.tile([P, CH], F32, tag="T0",
                                                  name=f"o0_{i}")
                                ps_o1 = ps_t.tile([P, CH], F32, tag="T1",
                                                  name=f"o1_{i}")
                                for d in range(KO):
                                    nc.tensor.matmul(
                                        ps_o0[:], tt_rd[:, d], wv_sb[:, d, 0:CH],
                                        start=(d == 0), stop=(d == KO - 1))
                                    nc.tensor.matmul(
                                        ps_o1[:], tt_rd[:, d], wv_sb[:, d, CH:D],
                                        start=(d == 0), stop=(d == KO - 1))
                                o_sb = dwork.tile([P, D], F32, tag="osb")
                                nc.vector.tensor_copy(o_sb[:, 0:CH], ps_o0[:])
                                nc.vector.tensor_copy(o_sb[:, CH:D], ps_o1[:])
                                nc.sync.dma_start(out[bass.ts(i, P), :], o_sb[:])

    nc.compile()
    _cached["nc"] = nc
    return nc


LAST_RESULT = None


def kernel(q, k, v, Wq, Wk, Wv, mask):
    global LAST_RESULT
    q = np.asarray(q, dtype=np.float32)
    k = np.asarray(k, dtype=np.float32)
    v = np.asarray(v, dtype=np.float32)
    Wq = np.asarray(Wq, dtype=np.float32)
    Wk = np.asarray(Wk, dtype=np.float32)
    Wv = np.asarray(Wv, dtype=np.float32)

    nc = _build()

    wq_s = np.ascontiguousarray(Wq / np.sqrt(np.float32(D)))
    wk_c = np.ascontiguousarray(Wk)
    wv_c = np.ascontiguousarray(Wv)
    ident = np.eye(P, dtype=np.float32)

    masks = []
    for h in range(2):
        m = np.zeros((P, 2, CH), dtype=np.float32)
        r = np.arange(P)[:, None]
        c = np.arange(CH)[None, :]
        for par in range(2):
            base = MASK_BASE[h][par]
            m[:, par, :] = np.where(c <= base + r, 0.0, NEG)
        masks.append(m)

    in_maps = []
    for core in range(8):
        b, h = core // 2, core % 2
        blocks = BLOCKS[h]
        qTb = q[b].T  # [D, S]
        cols = np.concatenate([np.arange(j * P, (j + 1) * P) for j in blocks])
        in_maps.append({
            "qT": np.ascontiguousarray(qTb[:, cols]),
            "kTh": np.ascontiguousarray(k[b].T[:, h * (S // 2):(h + 1) * (S // 2)]),
            "v": np.ascontiguousarray(v[b]),
            "wq": wq_s, "wk": wk_c, "wv": wv_c,
            "mask": masks[h], "ident": ident,
        })

    res = run_bass_kernel_spmd(nc, in_maps, list(range(8)),
                               trace=bool(os.environ.get("KERNEL_TRACE")))
    LAST_RESULT = res

    out = np.empty((B, S, D), dtype=np.float32)
    for core in range(8):
        b, h = core // 2, core % 2
        oc = np.asarray(res.results[core]["out"], dtype=np.float32)
        for pos, j in enumerate(BLOCKS[h]):
            out[b, j * P:(j + 1) * P, :] = oc[pos * P:(pos + 1) * P, :]
    return out


# revision 15
# speedup vs baseline: 1.3905x; 1.3905x over previous
"""Causal single-head attention (B=4, S=2048, D=1024) on 8 TRN2 NeuronCores.

Sharding: 2 cores per batch; each core owns 8 q-blocks of 128 rows chosen so
both cores of a batch see the same multiset of causal kv-span lengths
(padded to 512-chunks): core h=0 -> q-blocks [0,3,4,7,8,11,12,15],
core h=1 -> [1,2,5,6,9,10,13,14]; both give span chunks [1,1,2,2,3,3,4,4].
This makes one SPMD program valid for all 8 cores; per-core differences
(which q rows, causal mask offsets) ride in the input data.

Math per core (all matmuls in float32r, fp32 accumulation):
  Q^T = (Wq/sqrt(D))^T @ qT,  K^T = Wk^T @ kT        (projections)
  S_i = Q_i^T.T @ K^T (+ additive causal mask)        (scores per q-block)
  P = exp(S), denom = rowsum(P)                       (no max-sub: |S| < ~10)
  T_i = (P @ v) / denom                               (reassociated: raw v!)
  out_i = T_i @ Wv                                    (deferred out-proj)
Reassociation (P@v)@Wv replaces attn@(v@Wv) - saves the V projection.
"""

import os

import numpy as np

import concourse.bass as bass
import concourse.mybir as mybir
import concourse.tile as tile
from concourse import bacc
from concourse.bass_utils import run_bass_kernel_spmd

B, S, D = 4, 2048, 1024
P = 128                      # partitions / q-block rows
NBLK = 8                     # q-blocks per core
CH = 512                     # kv chunk (matmul moving free dim)
NCH = [1, 1, 2, 2, 3, 3, 4, 4]   # kv 512-chunks per q-block position
ORDER_A = [0, 2, 4, 5, 6, 7]    # first segment: needs all of v
ORDER_B = [3, 1]                # tail segment: only v chunks 0-7; frees SBUF
                                # so phase-4 inputs (wv, tt) prefetch under it
BLOCKS = [[0, 3, 4, 7, 8, 11, 12, 15], [1, 2, 5, 6, 9, 10, 13, 14]]
MASK_BASE = [[0, 384], [128, 256]]   # base[h][pos % 2]: col c allowed iff c <= base + r
DT = mybir.dt.float32r
F32 = mybir.dt.float32
NEG = -1e30

_cached = {}


def _build():
    if "nc" in _cached:
        return _cached["nc"]
    nc = bacc.Bacc("TRN2", target_bir_lowering=False, debug=False, num_devices=8)
    qT = nc.dram_tensor("qT", [D, P * NBLK], DT, kind="ExternalInput").ap()
    kT = nc.dram_tensor("kT", [D, S], DT, kind="ExternalInput").ap()
    v = nc.dram_tensor("v", [S, D], DT, kind="ExternalInput").ap()
    wq = nc.dram_tensor("wq", [D, D], DT, kind="ExternalInput").ap()
    wk = nc.dram_tensor("wk", [D, D], DT, kind="ExternalInput").ap()
    wv = nc.dram_tensor("wv", [D, D], DT, kind="ExternalInput").ap()
    mask = nc.dram_tensor("mask", [P, 2, CH], F32, kind="ExternalInput").ap()
    ident = nc.dram_tensor("ident", [P, P], DT, kind="ExternalInput").ap()
    out = nc.dram_tensor("out", [P * NBLK, D], F32, kind="ExternalOutput").ap()

    KO = D // P      # 8 contraction chunks
    NV = S // P      # 16 v row-chunks
    QK = S // 4      # kv quarter for kT staging

    kT_r = kT.rearrange("(ko p) s -> p ko s", p=P)

    with tile.TileContext(nc) as tc:
        with tc.tile_pool(name="pers", bufs=1) as pers, \
             tc.tile_pool(name="dram", bufs=1, space="DRAM") as dpool:
            ident_sb = pers.tile([P, P], DT)
            nc.sync.dma_start(ident_sb[:], ident)
            mask_sb = pers.tile([P, 2, CH], F32)
            nc.sync.dma_start(mask_sb[:], mask)
            QT_sb = pers.tile([P, KO, P * NBLK], DT)
            KT_sb = pers.tile([P, KO, S], DT)
            tt_dram = [dpool.tile([P, D], DT, name=f"ttd_{i}") for i in range(NBLK)]

            # ---- Phase 1: Q projection; Wk-half + first kT quarter prefetch ----
            wk_r = wk.rearrange("(ko p) m -> p ko m", p=P)
            kts = [None] * 4
            wk_h = [None] * 2

            def load_kt_quarter(pool, sq):
                t = pool.tile([P, KO, QK], DT, tag="kts", name=f"kts_{sq}")
                for ko in range(KO):
                    nc.sync.dma_start(
                        t[:, ko], kT_r[:, ko, sq * QK:(sq + 1) * QK])
                kts[sq] = t

            def load_wk_half(pool, hh):
                t = pool.tile([P, KO, D // 2], DT, name=f"wk_{hh}")
                for ko in range(KO):
                    nc.sync.dma_start(
                        t[:, ko], wk_r[:, ko, hh * (D // 2):(hh + 1) * (D // 2)])
                wk_h[hh] = t

            with tc.tile_pool(name="wkpool0", bufs=1) as wkpool0, \
                 tc.tile_pool(name="kstageA", bufs=1) as kstageA, \
                 tc.tile_pool(name="ps_proj", bufs=4, space="PSUM") as psp:
                with tc.tile_pool(name="qproj", bufs=1) as qpool:
                    qT_sb = qpool.tile([P, KO, P * NBLK], DT)
                    wq_sb = qpool.tile([P, KO, D], DT)
                    for ko in range(KO):
                        nc.sync.dma_start(
                            wq_sb[:, ko], wq.rearrange("(ko p) m -> p ko m", p=P)[:, ko])
                        nc.sync.dma_start(
                            qT_sb[:, ko], qT.rearrange("(ko p) s -> p ko s", p=P)[:, ko])
                    load_wk_half(wkpool0, 0)
                    load_kt_quarter(kstageA, 0)
                    for m in range(KO):
                        for n in range(2):
                            ps = psp.tile([P, CH], F32, tag="pp")
                            for k in range(KO):
                                nc.tensor.matmul(
                                    ps[:], wq_sb[:, k, bass.ts(m, P)],
                                    qT_sb[:, k, bass.ts(n, CH)],
                                    start=(k == 0), stop=(k == KO - 1))
                            nc.vector.tensor_copy(QT_sb[:, m, bass.ts(n, CH)], ps[:])

                # ---- Phase 2: K projection from quarter-staged kT ----
                with tc.tile_pool(name="wkpool1", bufs=1) as wkpool1, \
                     tc.tile_pool(name="kstageB", bufs=1) as kstageB:
                    load_wk_half(wkpool1, 1)
                    for sq in range(4):
                        if kts[sq] is None:
                            load_kt_quarter(kstageA if sq % 2 == 0 else kstageB, sq)
                        if sq + 1 < 4 and kts[sq + 1] is None:
                            load_kt_quarter(kstageA if (sq + 1) % 2 == 0 else kstageB,
                                            sq + 1)
                        for m in range(KO):
                            wk_t = wk_h[m // 4]
                            ps = psp.tile([P, CH], F32, tag="pp")
                            for k in range(KO):
                                nc.tensor.matmul(
                                    ps[:], wk_t[:, k, bass.ts(m % 4, P)],
                                    kts[sq][:, k, 0:CH],
                                    start=(k == 0), stop=(k == KO - 1))
                            nc.vector.tensor_copy(
                                KT_sb[:, m, bass.ds(sq * QK, CH)], ps[:])

            # ---- Phase 3: attention per q-block; T spilled to DRAM.
            #      Pipelined: scores run one chunk ahead of transpose+AV. ----
            wv_r = wv.rearrange("(ko p) m -> p ko m", p=P)
            v_r = v.rearrange("(so p) d -> p so d", p=P)
            if True:
                with tc.tile_pool(name="vlo", bufs=1) as vlo_pool, \
                     tc.tile_pool(name="cwork", bufs=2) as cwork, \
                     tc.tile_pool(name="ppool", bufs=3) as ppool, \
                     tc.tile_pool(name="ptpool", bufs=4) as ptpool, \
                     tc.tile_pool(name="ps_s", bufs=3, space="PSUM") as ps_s, \
                     tc.tile_pool(name="ps_tr", bufs=2, space="PSUM") as ps_tr, \
                     tc.tile_pool(name="ps_t", bufs=1, space="PSUM") as ps_t:
                    v_lo = vlo_pool.tile([P, NV // 2, D], DT)
                    for so in range(NV // 2):
                        nc.sync.dma_start(v_lo[:, so], v_r[:, so])

                    def v_chunk(kvi):
                        if kvi < NV // 2:
                            return v_lo[:, kvi]
                        return v_hi[:, kvi - NV // 2]

                    def attention_block(i):
                        nch = NCH[i]
                        nkv = nch * (CH // P)
                        ps_T0 = ps_t.tile([P, CH], F32, tag="T0",
                                          name=f"T0_{i}")
                        ps_T1 = ps_t.tile([P, CH], F32, tag="T1",
                                          name=f"T1_{i}")
                        dsums = []
                        p_tiles = []

                        def emit_scores(c, i=i, nch=nch):
                            ps_c = ps_s.tile([P, CH], F32, tag="s",
                                             name=f"s_{i}_{c}")
                            for k in range(KO):
                                nc.tensor.matmul(
                                    ps_c[:], QT_sb[:, k, bass.ts(i, P)],
                                    KT_sb[:, k, bass.ts(c, CH)],
                                    start=(k == 0), stop=(k == KO - 1))
                            if c == nch - 1:
                                nc.vector.tensor_tensor(
                                    ps_c[:], ps_c[:], mask_sb[:, i % 2],
                                    mybir.AluOpType.add)
                            p_sb = ppool.tile([P, CH], DT, tag="p",
                                              name=f"p_{i}_{c}")
                            ds = cwork.tile([P, 1], F32, tag="ds",
                                            name=f"ds_{i}_{c}")
                            nc.scalar.activation(
                                p_sb[:], ps_c[:],
                                mybir.ActivationFunctionType.Exp, accum_out=ds[:])
                            dsums.append(ds)
                            p_tiles.append(p_sb)

                        def emit_trav(c, i=i, nkv=nkv):
                            # transposes run 2 ahead of the AV matmuls
                            pts = []
                            for t in range(CH // P):
                                ptr = ps_tr.tile([P, P], DT, tag="tr")
                                nc.tensor.transpose(
                                    ptr[:], p_tiles[c][:, bass.ts(t, P)],
                                    ident_sb[:])
                                pt_sb = ptpool.tile([P, P], DT, tag="pt")
                                nc.vector.tensor_copy(pt_sb[:], ptr[:])
                                pts.append(pt_sb)
                                if t >= 2:
                                    _emit_av(c, t - 2, pts[t - 2], i, nkv)
                            _emit_av(c, 2, pts[2], i, nkv)
                            _emit_av(c, 3, pts[3], i, nkv)

                        def _emit_av(c, t, pt_sb, i, nkv):
                            kvi = c * (CH // P) + t
                            vc = v_chunk(kvi)
                            nc.tensor.matmul(
                                ps_T0[:], pt_sb[:], vc[:, 0:CH],
                                start=(kvi == 0), stop=(kvi == nkv - 1))
                            nc.tensor.matmul(
                                ps_T1[:], pt_sb[:], vc[:, CH:D],
                                start=(kvi == 0), stop=(kvi == nkv - 1))

                        for c in range(nch):
                            emit_scores(c)
                            if c >= 1:
                                emit_trav(c - 1)
                        emit_trav(nch - 1)

                        denom = cwork.tile([P, 1], F32, tag="den")
                        if nch == 1:
                            nc.vector.tensor_copy(denom[:], dsums[0][:])
                        else:
                            nc.vector.tensor_tensor(
                                denom[:], dsums[0][:], dsums[1][:],
                                mybir.AluOpType.add)
                            for c in range(2, nch):
                                nc.vector.tensor_tensor(
                                    denom[:], denom[:], dsums[c][:],
                                    mybir.AluOpType.add)
                        rden = cwork.tile([P, 1], F32, tag="rden")
                        nc.vector.reciprocal(rden[:], denom[:])
                        t_st = cwork.tile([P, D], DT, tag="tst", bufs=1)
                        nc.vector.tensor_scalar_mul(t_st[:, 0:CH], ps_T0[:], rden[:])
                        nc.vector.tensor_scalar_mul(t_st[:, CH:D], ps_T1[:], rden[:])
                        tt_st = cwork.tile([P, KO, P], DT, tag="ttst")
                        for d in range(KO):
                            ptr = ps_tr.tile([P, P], DT, tag="tr")
                            nc.tensor.transpose(
                                ptr[:], t_st[:, bass.ts(d, P)], ident_sb[:])
                            nc.vector.tensor_copy(tt_st[:, d], ptr[:])
                        nc.sync.dma_start(tt_dram[i][:], tt_st[:])

                    with tc.tile_pool(name="vhi", bufs=1) as vhi_pool:
                        v_hi = vhi_pool.tile([P, NV // 2, D], DT)
                        for so in range(NV // 2):
                            nc.sync.dma_start(v_hi[:, so], v_r[:, NV // 2 + so])
                        for i in ORDER_A:
                            attention_block(i)
                    # v_hi freed: prefetch phase-4 inputs under the tail blocks
                    with tc.tile_pool(name="wvpool", bufs=1) as wvpool:
                        wv_sb = wvpool.tile([P, KO, D], DT)
                        for ko in range(KO):
                            nc.sync.dma_start(wv_sb[:, ko], wv_r[:, ko])
                        for i in ORDER_B:
                            attention_block(i)

                        # ---- Phase 4: out = TT.T @ Wv (pure matmuls) ----
                        with tc.tile_pool(name="dwork", bufs=2) as dwork:
                            for i in range(NBLK):
                                tt_rd = dwork.tile([P, KO, P], DT, tag="ttrd")
                                nc.sync.dma_start(tt_rd[:], tt_dram[i][:])
                                ps_o0 = ps_t.tile([P, CH], F32, tag="T0",
                                                  name=f"o0_{i}")
                                ps_o1 = ps_t.tile([P, CH], F32, tag="T1",
                                                  name=f"o1_{i}")
                                for d in range(KO):
                                    nc.tensor.matmul(
                                        ps_o0[:], tt_rd[:, d], wv_sb[:, d, 0:CH],
                                        start=(d == 0), stop=(d == KO - 1))
                                    nc.tensor.matmul(
                                        ps_o1[:], tt_rd[:, d], wv_sb[:, d, CH:D],
                                        start=(d == 0), stop=(d == KO - 1))
                                o_sb = dwork.tile([P, D], F32, tag="osb")
                                nc.vector.tensor_copy(o_sb[:, 0:CH], ps_o0[:])
                                nc.vector.tensor_copy(o_sb[:, CH:D], ps_o1[:])
                                nc.sync.dma_start(out[bass.ts(i, P), :], o_sb[:])

    nc.compile()
    _cached["nc"] = nc
    return nc


LAST_RESULT = None


def kernel(q, k, v, Wq, Wk, Wv, mask):
    global LAST_RESULT
    q = np.asarray(q, dtype=np.float32)
    k = np.asarray(k, dtype=np.float32)
    v = np.asarray(v, dtype=np.float32)
    Wq = np.asarray(Wq, dtype=np.float32)
    Wk = np.asarray(Wk, dtype=np.float32)
    Wv = np.asarray(Wv, dtype=np.float32)

    nc = _build()

    wq_s = np.ascontiguousarray(Wq / np.sqrt(np.float32(D)))
    wk_c = np.ascontiguousarray(Wk)
    wv_c = np.ascontiguousarray(Wv)
    ident = np.eye(P, dtype=np.float32)

    masks = []
    for h in range(2):
        m = np.zeros((P, 2, CH), dtype=np.float32)
        r = np.arange(P)[:, None]
        c = np.arange(CH)[None, :]
        for par in range(2):
            base = MASK_BASE[h][par]
            m[:, par, :] = np.where(c <= base + r, 0.0, NEG)
        masks.append(m)

    in_maps = []
    for core in range(8):
        b, h = core // 2, core % 2
        blocks = BLOCKS[h]
        qTb = q[b].T  # [D, S]
        cols = np.concatenate([np.arange(j * P, (j + 1) * P) for j in blocks])
        in_maps.append({
            "qT": np.ascontiguousarray(qTb[:, cols]),
            "kT": np.ascontiguousarray(k[b].T),
            "v": np.ascontiguousarray(v[b]),
            "wq": wq_s, "wk": wk_c, "wv": wv_c,
            "mask": masks[h], "ident": ident,
        })

    res = run_bass_kernel_spmd(nc, in_maps, list(range(8)),
                               trace=bool(os.environ.get("KERNEL_TRACE")))
    LAST_RESULT = res

    out = np.empty((B, S, D), dtype=np.float32)
    for core in range(8):
        b, h = core // 2, core % 2
        oc = np.asarray(res.results[core]["out"], dtype=np.float32)
        for pos, j in enumerate(BLOCKS[h]):
            out[b, j * P:(j + 1) * P, :] = oc[pos * P:(pos + 1) * P, :]
    return out


# revision 17
# speedup vs baseline: 1.5117x; 1.0871x over previous
"""Causal single-head attention (B=4, S=2048, D=1024) on 8 TRN2 NeuronCores.

Sharding: 2 cores per batch; each core owns 8 q-blocks of 128 rows chosen so
both cores of a batch see the same multiset of causal kv-span lengths
(padded to 512-chunks): core h=0 -> q-blocks [0,3,4,7,8,11,12,15],
core h=1 -> [1,2,5,6,9,10,13,14]; both give span chunks [1,1,2,2,3,3,4,4].
This makes one SPMD program valid for all 8 cores; per-core differences
(which q rows, causal mask offsets) ride in the input data.

Math per core (all matmuls in float32r, fp32 accumulation):
  Q^T = (Wq/sqrt(D))^T @ qT,  K^T = Wk^T @ kT        (projections)
  S_i = Q_i^T.T @ K^T (+ additive causal mask)        (scores per q-block)
  P = exp(S), denom = rowsum(P)                       (no max-sub: |S| < ~10)
  T_i = (P @ v) / denom                               (reassociated: raw v!)
  out_i = T_i @ Wv                                    (deferred out-proj)
Reassociation (P@v)@Wv replaces attn@(v@Wv) - saves the V projection.
"""

import os

import numpy as np

import concourse.bass as bass
import concourse.mybir as mybir
import concourse.tile as tile
from concourse import bacc
from concourse.bass_utils import run_bass_kernel_spmd

B, S, D = 4, 2048, 1024
P = 128                      # partitions / q-block rows
NBLK = 8                     # q-blocks per core
CH = 512                     # kv chunk (matmul moving free dim)
NCH = [1, 1, 2, 2, 3, 3, 4, 4]   # kv 512-chunks per q-block position
ORDER_A = [0, 2, 4, 5, 6, 7]    # first segment: needs all of v
ORDER_B = [3, 1]                # tail segment: only v chunks 0-7; frees SBUF
                                # so phase-4 inputs (wv, tt) prefetch under it
BLOCKS = [[0, 3, 4, 7, 8, 11, 12, 15], [1, 2, 5, 6, 9, 10, 13, 14]]
MASK_BASE = [[0, 384], [128, 256]]   # base[h][pos % 2]: col c allowed iff c <= base + r
DT = mybir.dt.float32r
F32 = mybir.dt.float32
NEG = -1e30

_cached = {}


def _build():
    if "nc" in _cached:
        return _cached["nc"]
    nc = bacc.Bacc("TRN2", target_bir_lowering=False, debug=False, num_devices=8)
    qT = nc.dram_tensor("qT", [D, P * NBLK], DT, kind="ExternalInput").ap()
    kT = nc.dram_tensor("kT", [D, S], DT, kind="ExternalInput").ap()
    v = nc.dram_tensor("v", [S, D], DT, kind="ExternalInput").ap()
    wq = nc.dram_tensor("wq", [D, D], DT, kind="ExternalInput").ap()
    wk = nc.dram_tensor("wk", [D, D], DT, kind="ExternalInput").ap()
    wv = nc.dram_tensor("wv", [D, D], DT, kind="ExternalInput").ap()
    mask = nc.dram_tensor("mask", [P, 2, CH], F32, kind="ExternalInput").ap()
    ident = nc.dram_tensor("ident", [P, P], DT, kind="ExternalInput").ap()
    out = nc.dram_tensor("out", [P * NBLK, D], F32, kind="ExternalOutput").ap()

    KO = D // P      # 8 contraction chunks
    NV = S // P      # 16 v row-chunks
    QK = S // 4      # kv quarter for kT staging

    kT_r = kT.rearrange("(ko p) s -> p ko s", p=P)

    with tile.TileContext(nc) as tc:
        with tc.tile_pool(name="pers", bufs=1) as pers, \
             tc.tile_pool(name="dram", bufs=1, space="DRAM") as dpool:
            ident_sb = pers.tile([P, P], DT)
            nc.sync.dma_start(ident_sb[:], ident)
            mask_sb = pers.tile([P, 2, CH], F32)
            nc.sync.dma_start(mask_sb[:], mask)
            QT_sb = pers.tile([P, KO, P * NBLK], DT)
            KT_sb = pers.tile([P, KO, S], DT)
            tt_dram = [dpool.tile([P, D], DT, name=f"ttd_{i}") for i in range(NBLK)]

            # ---- Phase 1: Q projection; Wk-half + first kT quarter prefetch ----
            wk_r = wk.rearrange("(ko p) m -> p ko m", p=P)
            kts = [None] * 4
            wk_h = [None] * 2

            def load_kt_quarter(pool, sq):
                t = pool.tile([P, KO, QK], DT, tag="kts", name=f"kts_{sq}")
                for ko in range(KO):
                    nc.sync.dma_start(
                        t[:, ko], kT_r[:, ko, sq * QK:(sq + 1) * QK])
                kts[sq] = t

            def load_wk_half(pool, hh):
                t = pool.tile([P, KO, D // 2], DT, name=f"wk_{hh}")
                for ko in range(KO):
                    nc.sync.dma_start(
                        t[:, ko], wk_r[:, ko, hh * (D // 2):(hh + 1) * (D // 2)])
                wk_h[hh] = t

            with tc.tile_pool(name="wkpool0", bufs=1) as wkpool0, \
                 tc.tile_pool(name="kstageA", bufs=1) as kstageA, \
                 tc.tile_pool(name="ps_proj", bufs=4, space="PSUM") as psp:
                with tc.tile_pool(name="qproj", bufs=1) as qpool:
                    qT_sb = qpool.tile([P, KO, P * NBLK], DT)
                    wq_sb = qpool.tile([P, KO, D], DT)
                    wq_r2 = wq.rearrange("(ko p) m -> p ko m", p=P)
                    for ko in range(KO):
                        nc.sync.dma_start(wq_sb[:, ko, 0:D // 2],
                                          wq_r2[:, ko, 0:D // 2])
                        nc.sync.dma_start(
                            qT_sb[:, ko], qT.rearrange("(ko p) s -> p ko s", p=P)[:, ko])
                    for ko in range(KO):
                        nc.sync.dma_start(wq_sb[:, ko, D // 2:D],
                                          wq_r2[:, ko, D // 2:D])
                    load_wk_half(wkpool0, 0)
                    load_kt_quarter(kstageA, 0)
                    for m in range(KO):
                        for n in range(2):
                            ps = psp.tile([P, CH], F32, tag="pp")
                            for k in range(KO):
                                nc.tensor.matmul(
                                    ps[:], wq_sb[:, k, bass.ts(m, P)],
                                    qT_sb[:, k, bass.ts(n, CH)],
                                    start=(k == 0), stop=(k == KO - 1))
                            nc.vector.tensor_copy(QT_sb[:, m, bass.ts(n, CH)], ps[:])

                # ---- Phase 2: K projection from quarter-staged kT ----
                with tc.tile_pool(name="wkpool1", bufs=1) as wkpool1, \
                     tc.tile_pool(name="kstageB", bufs=1) as kstageB:
                    load_wk_half(wkpool1, 1)
                    for sq in range(4):
                        if kts[sq] is None:
                            load_kt_quarter(kstageA if sq % 2 == 0 else kstageB, sq)
                        if sq + 1 < 4 and kts[sq + 1] is None:
                            load_kt_quarter(kstageA if (sq + 1) % 2 == 0 else kstageB,
                                            sq + 1)
                        for m in range(KO):
                            wk_t = wk_h[m // 4]
                            ps = psp.tile([P, CH], F32, tag="pp")
                            for k in range(KO):
                                nc.tensor.matmul(
                                    ps[:], wk_t[:, k, bass.ts(m % 4, P)],
                                    kts[sq][:, k, 0:CH],
                                    start=(k == 0), stop=(k == KO - 1))
                            nc.vector.tensor_copy(
                                KT_sb[:, m, bass.ds(sq * QK, CH)], ps[:])

            # ---- Phase 3: attention per q-block; T spilled to DRAM.
            #      Pipelined: scores run one chunk ahead of transpose+AV. ----
            wv_r = wv.rearrange("(ko p) m -> p ko m", p=P)
            v_r = v.rearrange("(so p) d -> p so d", p=P)
            if True:
                with tc.tile_pool(name="vlo", bufs=1) as vlo_pool, \
                     tc.tile_pool(name="cwork", bufs=2) as cwork, \
                     tc.tile_pool(name="ppool", bufs=3) as ppool, \
                     tc.tile_pool(name="ptpool", bufs=4) as ptpool, \
                     tc.tile_pool(name="ps_s", bufs=3, space="PSUM") as ps_s, \
                     tc.tile_pool(name="ps_tr", bufs=2, space="PSUM") as ps_tr, \
                     tc.tile_pool(name="ps_t", bufs=1, space="PSUM") as ps_t:
                    v_lo = vlo_pool.tile([P, NV // 2, D], DT)
                    for so in range(NV // 2):
                        nc.sync.dma_start(v_lo[:, so], v_r[:, so])

                    def v_chunk(kvi):
                        if kvi < NV // 2:
                            return v_lo[:, kvi]
                        return v_hi[:, kvi - NV // 2]

                    def attention_block(i):
                        nch = NCH[i]
                        nkv = nch * (CH // P)
                        ps_T0 = ps_t.tile([P, CH], F32, tag="T0",
                                          name=f"T0_{i}")
                        ps_T1 = ps_t.tile([P, CH], F32, tag="T1",
                                          name=f"T1_{i}")
                        dsums = []
                        p_tiles = []

                        def emit_scores(c, i=i, nch=nch):
                            ps_c = ps_s.tile([P, CH], F32, tag="s",
                                             name=f"s_{i}_{c}")
                            for k in range(KO):
                                nc.tensor.matmul(
                                    ps_c[:], QT_sb[:, k, bass.ts(i, P)],
                                    KT_sb[:, k, bass.ts(c, CH)],
                                    start=(k == 0), stop=(k == KO - 1))
                            if c == nch - 1:
                                nc.vector.tensor_tensor(
                                    ps_c[:], ps_c[:], mask_sb[:, i % 2],
                                    mybir.AluOpType.add)
                            p_sb = ppool.tile([P, CH], DT, tag="p",
                                              name=f"p_{i}_{c}")
                            ds = cwork.tile([P, 1], F32, tag="ds",
                                            name=f"ds_{i}_{c}")
                            nc.scalar.activation(
                                p_sb[:], ps_c[:],
                                mybir.ActivationFunctionType.Exp, accum_out=ds[:])
                            dsums.append(ds)
                            p_tiles.append(p_sb)

                        def emit_trav(c, i=i, nkv=nkv):
                            # transposes run 2 ahead of the AV matmuls
                            pts = []
                            for t in range(CH // P):
                                ptr = ps_tr.tile([P, P], DT, tag="tr")
                                nc.tensor.transpose(
                                    ptr[:], p_tiles[c][:, bass.ts(t, P)],
                                    ident_sb[:])
                                pt_sb = ptpool.tile([P, P], DT, tag="pt")
                                nc.vector.tensor_copy(pt_sb[:], ptr[:])
                                pts.append(pt_sb)
                                if t >= 2:
                                    _emit_av(c, t - 2, pts[t - 2], i, nkv)
                            _emit_av(c, 2, pts[2], i, nkv)
                            _emit_av(c, 3, pts[3], i, nkv)

                        def _emit_av(c, t, pt_sb, i, nkv):
                            kvi = c * (CH // P) + t
                            vc = v_chunk(kvi)
                            nc.tensor.matmul(
                                ps_T0[:], pt_sb[:], vc[:, 0:CH],
                                start=(kvi == 0), stop=(kvi == nkv - 1))
                            nc.tensor.matmul(
                                ps_T1[:], pt_sb[:], vc[:, CH:D],
                                start=(kvi == 0), stop=(kvi == nkv - 1))

                        for c in range(nch):
                            emit_scores(c)
                            if c >= 1:
                                emit_trav(c - 1)
                        emit_trav(nch - 1)

                        denom = cwork.tile([P, 1], F32, tag="den")
                        if nch == 1:
                            nc.vector.tensor_copy(denom[:], dsums[0][:])
                        else:
                            nc.vector.tensor_tensor(
                                denom[:], dsums[0][:], dsums[1][:],
                                mybir.AluOpType.add)
                            for c in range(2, nch):
                                nc.vector.tensor_tensor(
                                    denom[:], denom[:], dsums[c][:],
                                    mybir.AluOpType.add)
                        rden = cwork.tile([P, 1], F32, tag="rden")
                        nc.vector.reciprocal(rden[:], denom[:])
                        t_st = cwork.tile([P, D], DT, tag="tst", bufs=1)
                        nc.vector.tensor_scalar_mul(t_st[:, 0:CH], ps_T0[:], rden[:])
                        nc.vector.tensor_scalar_mul(t_st[:, CH:D], ps_T1[:], rden[:])
                        tt_st = cwork.tile([P, KO, P], DT, tag="ttst")
                        for d in range(KO):
                            ptr = ps_tr.tile([P, P], DT, tag="tr")
                            nc.tensor.transpose(
                                ptr[:], t_st[:, bass.ts(d, P)], ident_sb[:])
                            nc.vector.tensor_copy(tt_st[:, d], ptr[:])
                        nc.sync.dma_start(tt_dram[i][:], tt_st[:])

                    with tc.tile_pool(name="vhi", bufs=1) as vhi_pool:
                        v_hi = vhi_pool.tile([P, NV // 2, D], DT)
                        for so in range(NV // 2):
                            nc.sync.dma_start(v_hi[:, so], v_r[:, NV // 2 + so])
                        for i in ORDER_A:
                            attention_block(i)
                    # v_hi freed: prefetch phase-4 inputs under the tail blocks
                    with tc.tile_pool(name="wvpool", bufs=1) as wvpool, \
                         tc.tile_pool(name="dwork", bufs=3) as dwork, \
                         tc.tile_pool(name="owork", bufs=2) as owork:
                        wv_sb = wvpool.tile([P, KO, D], DT)
                        for ko in range(KO):
                            nc.sync.dma_start(wv_sb[:, ko], wv_r[:, ko])
                        tt_rds = {}
                        for i in ORDER_A[:3]:
                            tt_rds[i] = dwork.tile([P, KO, P], DT, tag="ttrd",
                                                   name=f"ttrd_{i}")
                            nc.sync.dma_start(tt_rds[i][:], tt_dram[i][:])
                        for i in ORDER_B:
                            attention_block(i)

                        # ---- Phase 4: out = TT.T @ Wv (pure matmuls) ----
                        for step in range(NBLK):
                            i = (ORDER_A + ORDER_B)[step]
                            if i in tt_rds:
                                tt_rd = tt_rds.pop(i)
                            else:
                                tt_rd = dwork.tile([P, KO, P], DT, tag="ttrd",
                                                   name=f"ttrd_{i}")
                                nc.sync.dma_start(tt_rd[:], tt_dram[i][:])
                            if step % 2 == 0:
                                ps_o0 = ps_t.tile([P, CH], F32, tag="T0",
                                                  name=f"o0_{i}")
                                ps_o1 = ps_t.tile([P, CH], F32, tag="T1",
                                                  name=f"o1_{i}")
                            else:
                                ps_o0 = ps_tr.tile([P, CH], F32, tag="tr",
                                                   name=f"o0_{i}")
                                ps_o1 = ps_tr.tile([P, CH], F32, tag="tr",
                                                   name=f"o1_{i}")
                            for d in range(KO):
                                nc.tensor.matmul(
                                    ps_o0[:], tt_rd[:, d], wv_sb[:, d, 0:CH],
                                    start=(d == 0), stop=(d == KO - 1))
                                nc.tensor.matmul(
                                    ps_o1[:], tt_rd[:, d], wv_sb[:, d, CH:D],
                                    start=(d == 0), stop=(d == KO - 1))
                            o_sb = owork.tile([P, D], F32, tag="osb")
                            nc.vector.tensor_copy(o_sb[:, 0:CH], ps_o0[:])
                            nc.vector.tensor_copy(o_sb[:, CH:D], ps_o1[:])
                            nc.sync.dma_start(out[bass.ts(i, P), :], o_sb[:])

    nc.compile()
    _cached["nc"] = nc
    return nc


LAST_RESULT = None


def kernel(q, k, v, Wq, Wk, Wv, mask):
    global LAST_RESULT
    q = np.asarray(q, dtype=np.float32)
    k = np.asarray(k, dtype=np.float32)
    v = np.asarray(v, dtype=np.float32)
    Wq = np.asarray(Wq, dtype=np.float32)
    Wk = np.asarray(Wk, dtype=np.float32)
    Wv = np.asarray(Wv, dtype=np.float32)

    nc = _build()

    wq_s = np.ascontiguousarray(Wq / np.sqrt(np.float32(D)))
    wk_c = np.ascontiguousarray(Wk)
    wv_c = np.ascontiguousarray(Wv)
    ident = np.eye(P, dtype=np.float32)

    masks = []
    for h in range(2):
        m = np.zeros((P, 2, CH), dtype=np.float32)
        r = np.arange(P)[:, None]
        c = np.arange(CH)[None, :]
        for par in range(2):
            base = MASK_BASE[h][par]
            m[:, par, :] = np.where(c <= base + r, 0.0, NEG)
        masks.append(m)

    in_maps = []
    for core in range(8):
        b, h = core // 2, core % 2
        blocks = BLOCKS[h]
        qTb = q[b].T  # [D, S]
        cols = np.concatenate([np.arange(j * P, (j + 1) * P) for j in blocks])
        in_maps.append({
            "qT": np.ascontiguousarray(qTb[:, cols]),
            "kT": np.ascontiguousarray(k[b].T),
            "v": np.ascontiguousarray(v[b]),
            "wq": wq_s, "wk": wk_c, "wv": wv_c,
            "mask": masks[h], "ident": ident,
        })

    res = run_bass_kernel_spmd(nc, in_maps, list(range(8)),
                               trace=bool(os.environ.get("KERNEL_TRACE")))
    LAST_RESULT = res

    out = np.empty((B, S, D), dtype=np.float32)
    for core in range(8):
        b, h = core // 2, core % 2
        oc = np.asarray(res.results[core]["out"], dtype=np.float32)
        for pos, j in enumerate(BLOCKS[h]):
            out[b, j * P:(j + 1) * P, :] = oc[pos * P:(pos + 1) * P, :]
    return out
